# revision 1
# baseline (speedup 1.0000x reference)
"""Bahdanau attention kernel for Trainium2 (Bass/Tile), data-parallel over batch.

Problem (full shapes):
    encoder_output   [S=2048, B=16, H=1024] f32
    last_decoder_state [2, 1, B, H] f32   (only [0,0] used -> state [B, H])
    W [H, H], b [H]
    energy = state @ W.T + b                  [B, H]
    scores = einsum('sbh,bh->sb', enc, energy) [S, B]
    out    = softmax(scores, axis=0)[None, None]  [1, 1, S, B]

Sharding: batch split across 8 cores (2 batches each); W/b replicated.
Softmax is over S which is fully resident per core -> no collectives.

Per-core device program:
    energy[j, b] = sum_i W[j,i] * state[b,i] + bias[j]   (PE, W^T streamed)
    scores[b, s] = sum_h energy[h, b] * enc[b, h, s]     (PE matvec, PSUM accum)
    probs = softmax over s                                (vector/scalar engines)

Host-side prep: slice per-core batches, transpose enc slice to [b, h, s] and W
to W^T so every DMA reads long contiguous rows.

`reps`/`dynamic` exist only for benchmarking: they repeat the body inside one
NEFF (statically unrolled or as a Tile For_i loop) so HW time can be measured
through a high-latency dispatch path. kernel() always uses reps=1.
"""

import numpy as np

S, B, H = 2048, 16, 1024
NCORES = 8
BL = B // NCORES  # 2 batches per core
P = 128           # partitions
HT = H // P       # 8 h-tiles
SCW = 512         # matmul moving-operand max (one PSUM bank of f32)
SC = S // SCW     # 4 seq chunks

_cached = {}


def _build_nc(reps=1, dynamic=False):
    import concourse.bacc as bacc
    import concourse.bass as bass
    import concourse.tile as tile
    from concourse import mybir

    f32 = mybir.dt.float32
    nc = bacc.Bacc("TRN2", target_bir_lowering=False, debug=False, num_devices=NCORES)

    enc = nc.dram_tensor("enc", [BL, H, S], f32, kind="ExternalInput").ap()
    state_t = nc.dram_tensor("state_t", [H, BL], f32, kind="ExternalInput").ap()
    w_t = nc.dram_tensor("w_t", [H, H], f32, kind="ExternalInput").ap()
    bias = nc.dram_tensor("bias", [H], f32, kind="ExternalInput").ap()
    probs = nc.dram_tensor("probs", [BL, S], f32, kind="ExternalOutput").ap()

    with tile.TileContext(nc) as tc:
        with (
            tc.tile_pool(name="consts", bufs=min(2, reps)) as consts,
            tc.tile_pool(name="wpool", bufs=HT) as wpool,
            tc.tile_pool(name="encpool", bufs=4) as encpool,
            tc.tile_pool(name="pe_ps", bufs=2, space=bass.MemorySpace.PSUM) as pe_pool,
            tc.tile_pool(name="sc_ps", bufs=1, space=bass.MemorySpace.PSUM) as ps_pool,
            tc.tile_pool(name="spool", bufs=min(2, reps)) as spool,
        ):

            def emit_rep():
                # W/state/bias ride the SWDGE stream so the two HWDGE rings
                # are free for the big encoder reads (three concurrent DMA
                # streams round-robin at packet granularity on the SDMAs)
                # state^T [i on partitions, (i_tile, b) free]
                st = consts.tile([P, HT, BL], f32)
                nc.gpsimd.dma_start(
                    out=st[:], in_=state_t.rearrange("(t p) b -> p t b", p=P)
                )
                # bias [j on partitions, j_tile free]
                bt = consts.tile([P, HT], f32)
                nc.gpsimd.dma_start(out=bt[:], in_=bias.rearrange("(t p) -> p t", p=P))

                # W^T i-tiles: [128 i, 1024 j] each, 4KB contiguous rows
                wts = []
                for it in range(HT):
                    wt = wpool.tile([P, H], f32)
                    nc.gpsimd.dma_start(out=wt[:], in_=w_t[it * P:(it + 1) * P, :])
                    wts.append(wt)

                # energy[j % 128, j_tile, b] = sum_i W[j,i] state[b,i] + bias[j]
                energy = consts.tile([P, HT, BL], f32)
                for jt in range(HT):
                    pe = pe_pool.tile([P, BL], f32)
                    for it in range(HT):
                        nc.tensor.matmul(
                            pe[:],
                            wts[it][:, jt * P:(jt + 1) * P],  # lhsT [i, j]
                            st[:, it, :],                     # rhs  [i, b]
                            start=(it == 0),
                            stop=(it == HT - 1),
                        )
                    nc.scalar.activation(
                        out=energy[:, jt, :],
                        in_=pe[:],
                        func=mybir.ActivationFunctionType.Identity,
                        bias=bt[:, jt:jt + 1],
                        scale=1.0,
                    )

                # scores psum: b=0 at partition 0, b=1 at partition 32 (matmul
                # output base partition must be 0/32/64), s on free dim
                ps = ps_pool.tile([33, S], f32)
                hwdge = [nc.sync, nc.scalar]  # two independent HWDGE rings
                for ht in range(HT):
                    # one 2 MB transfer per h-tile covering both batches:
                    # halves the per-transfer fixed cost on the rings
                    et = encpool.tile([P, BL, S], f32)
                    hwdge[ht % 2].dma_start(
                        out=et[:],
                        in_=enc[:, ht * P:(ht + 1) * P, :].rearrange(
                            "b p s -> p b s"
                        ),
                    )
                    for b in range(BL):
                        for sc in range(SC):
                            nc.tensor.matmul(
                                ps[32 * b:32 * b + 1, sc * SCW:(sc + 1) * SCW],
                                energy[:, ht, b:b + 1],             # lhsT [h, 1]
                                et[:, b, sc * SCW:(sc + 1) * SCW],  # rhs [h, s]
                                start=(ht == 0),
                                stop=(ht == HT - 1),
                            )

                # softmax over s (free dim), per batch; every op's APs share
                # the same base partition (32*b) so engine lanes stay aligned
                prob_sb = spool.tile([33, S], f32)
                nmax = spool.tile([33, 1], f32)
                ssum = spool.tile([33, 1], f32)
                rinv = spool.tile([33, 1], f32)
                for b in range(BL):
                    r = 32 * b
                    nc.vector.reduce_max(
                        nmax[r:r + 1, :], ps[r:r + 1, :],
                        axis=mybir.AxisListType.X, negate=True,
                    )
                    nc.scalar.activation(
                        out=prob_sb[r:r + 1, :],
                        in_=ps[r:r + 1, :],
                        func=mybir.ActivationFunctionType.Exp,
                        bias=nmax[r:r + 1, :],
                        scale=1.0,
                        accum_out=ssum[r:r + 1, :],
                    )
                    nc.vector.reciprocal(rinv[r:r + 1, :], ssum[r:r + 1, :])
                    nc.vector.tensor_scalar_mul(
                        out=prob_sb[r:r + 1, :],
                        in0=prob_sb[r:r + 1, :],
                        scalar1=rinv[r:r + 1, :],
                    )
                    nc.sync.dma_start(
                        out=probs[b:b + 1, :], in_=prob_sb[r:r + 1, :]
                    )

            if dynamic and reps > 1:
                with tc.For_i(0, reps, 1):
                    emit_rep()
            else:
                for _rep in range(reps):
                    emit_rep()

    nc.compile()
    return nc


def get_nc(reps=1, dynamic=False):
    key = ("nc", reps, dynamic)
    if key not in _cached:
        _cached[key] = _build_nc(reps, dynamic)
    return _cached[key]


def prep_in_maps(encoder_output, last_decoder_state, W, b):
    enc = np.asarray(encoder_output, dtype=np.float32)
    state = np.asarray(last_decoder_state, dtype=np.float32)[0, 0]  # [B, H]
    Wt = np.ascontiguousarray(np.asarray(W, dtype=np.float32).T)    # [i, j]
    bias = np.ascontiguousarray(np.asarray(b, dtype=np.float32))
    in_maps = []
    for c in range(NCORES):
        b0 = BL * c
        in_maps.append({
            "enc": np.ascontiguousarray(enc[:, b0:b0 + BL, :].transpose(1, 2, 0)),
            "state_t": np.ascontiguousarray(state[b0:b0 + BL, :].T),
            "w_t": Wt,
            "bias": bias,
        })
    return in_maps


def assemble(results):
    out = np.empty((S, B), np.float32)
    for c in range(NCORES):
        out[:, BL * c:BL * (c + 1)] = results[c]["probs"].T
    return out[None, None]


def kernel(encoder_output, last_decoder_state, W, b):
    from concourse.bass_utils import run_bass_kernel_spmd

    nc = get_nc()
    in_maps = prep_in_maps(encoder_output, last_decoder_state, W, b)
    res = run_bass_kernel_spmd(nc, in_maps, core_ids=list(range(NCORES)))
    return assemble(res.results)



# revision 2
# speedup vs baseline: 3.7777x; 3.7777x over previous
"""Bahdanau attention kernel for Trainium2 (Bass/Tile), data-parallel over batch.

Problem (full shapes):
    encoder_output   [S=2048, B=16, H=1024] f32
    last_decoder_state [2, 1, B, H] f32   (only [0,0] used -> state [B, H])
    W [H, H], b [H]
    energy = state @ W.T + b                  [B, H]
    scores = einsum('sbh,bh->sb', enc, energy) [S, B]
    out    = softmax(scores, axis=0)[None, None]  [1, 1, S, B]

Sharding: batch split across 8 cores (2 batches each); W/b replicated.
Softmax is over S which is fully resident per core -> no collectives.

The kernel is DMA-bound (memory regime): per core it must stream its
16.8 MB enc slice + 4 MB W per rep in f32.  Both are cast to fp16 on the
host (validated: final rel err 6.8e-3 vs the 2e-2 gate; bf16 fails at
5.4e-2), halving HBM traffic to ~10.4 MB -> ~29 us roofline at 358 GB/s.

Per-core device program (all host-side layouts are DMA-natural, every
descriptor is a contiguous 2-8 KB run):
    energy[j, b] = sum_i W[j,i] state[b,i] + bias[j]   (PE, W^T j-blocks)
    scores[b, s] = sum_h energy[h, b] enc[b, h, s]     (PE, fp16, f32 PSUM
        accum; the two batches run concurrently in separate 32-col groups
        via tile_position)
    probs = softmax over s                             (joint [33,*] ops:
        both batches (partitions 0/32) in one instruction per stage)

All input DMAs ride one HWDGE ring (sync engine) in FIFO order,
W j-blocks interleaved with enc h-tiles so the energy->scores pipeline
starts immediately; the output DMA rides the scalar ring so a rep's tail
store can never block the next rep's input stream.

`reps`/`dynamic` exist only for benchmarking: they repeat the body inside
one NEFF so HW time can be measured through a high-latency dispatch path.
The dynamic form amortizes For_i's all-engine barrier over UNROLL body
copies per iteration, and the tile pools double-buffer across body copies
so reps pipeline (steady-state throughput, DMA-limited). kernel() always
uses reps=1.
"""

import numpy as np

S, B, H = 2048, 16, 1024
NCORES = 8
BL = B // NCORES  # 2 batches per core
P = 128           # partitions
HT = H // P       # 8 h-tiles
SCW = 512         # matmul moving-operand chunk (one PSUM bank of f32)
SC = S // SCW     # 4 seq chunks
UNROLL = 8        # body copies per For_i iteration in dynamic bench mode

_cached = {}


def _build_nc(reps=1, dynamic=False):
    import concourse.bacc as bacc
    import concourse.bass as bass
    import concourse.tile as tile
    from concourse import mybir

    f16 = mybir.dt.float16
    f32 = mybir.dt.float32
    nc = bacc.Bacc("TRN2", target_bir_lowering=False, debug=False, num_devices=NCORES)

    # host-prepped layouts (see prep_in_maps):
    # enc_t[ht, p, bl, s] = enc[s, b0+bl, ht*128+p]          fp16
    # wtb[jt, p, it, j]   = W[jt*128+j, it*128+p]            fp16
    # st_in[p, it, bl]    = state[b0+bl, it*128+p]           fp16
    # bt_in[p, jt]        = bias[jt*128+p]                   f32
    enc_t = nc.dram_tensor("enc_t", [HT, P, BL, S], f16, kind="ExternalInput").ap()
    wtb = nc.dram_tensor("wtb", [HT, P, HT, P], f16, kind="ExternalInput").ap()
    st_in = nc.dram_tensor("st_in", [P, HT, BL], f16, kind="ExternalInput").ap()
    bt_in = nc.dram_tensor("bt_in", [P, HT], f32, kind="ExternalInput").ap()
    probs = nc.dram_tensor("probs", [BL, S], f32, kind="ExternalOutput").ap()

    with tile.TileContext(nc) as tc:
        with (
            tc.tile_pool(name="consts", bufs=2) as consts,
            tc.tile_pool(name="wpool", bufs=3) as wpool,
            tc.tile_pool(name="encpool", bufs=3) as encpool,
            tc.tile_pool(name="pe_ps", bufs=2, space=bass.MemorySpace.PSUM) as pe_pool,
            tc.tile_pool(name="sc_ps", bufs=1, space=bass.MemorySpace.PSUM) as ps_pool,
            tc.tile_pool(name="spool", bufs=2) as spool,
        ):

            def emit_rep():
                st = consts.tile([P, HT, BL], f16)
                nc.sync.dma_start(out=st[:], in_=st_in)
                bt = consts.tile([P, HT], f32)
                nc.sync.dma_start(out=bt[:], in_=bt_in)

                # input stream, one FIFO ring: w-blocks early + interleaved
                # so energy[jt] is ready long before et[ht=jt] lands
                wts = [None] * HT
                ets = [None] * HT

                def load_w(jt):
                    wt = wpool.tile([P, HT, P], f16)
                    nc.sync.dma_start(out=wt[:], in_=wtb[jt])
                    wts[jt] = wt

                def load_e(ht):
                    et = encpool.tile([P, BL, S], f16)
                    nc.sync.dma_start(out=et[:], in_=enc_t[ht])
                    ets[ht] = et

                load_w(0); load_w(1); load_e(0)
                load_w(2); load_w(3); load_e(1)
                load_w(4); load_w(5); load_e(2)
                load_w(6); load_w(7); load_e(3)
                load_e(4); load_e(5); load_e(6); load_e(7)

                # energy[j % 128, jt, b] = sum_i W[j,i] state[b,i] + bias[j]
                energy = consts.tile([P, HT, BL], f16)
                for jt in range(HT):
                    pe = pe_pool.tile([P, BL], f32)
                    for it in range(HT):
                        nc.tensor.matmul(
                            pe[:],
                            wts[jt][:, it, :],  # lhsT [i, j]
                            st[:, it, :],       # rhs  [i, b]
                            start=(it == 0),
                            stop=(it == HT - 1),
                        )
                    nc.vector.tensor_scalar_add(
                        out=energy[:, jt, :], in0=pe[:], scalar1=bt[:, jt:jt + 1]
                    )

                # scores psum: b=0 on partition 0, b=1 on partition 32; the
                # two batches run concurrently in separate 32-col groups
                ps = ps_pool.tile([33, S], f32)
                for ht in range(HT):
                    for sc in range(SC):
                        for b in range(BL):
                            nc.tensor.matmul(
                                ps[32 * b:32 * b + 1, sc * SCW:(sc + 1) * SCW],
                                energy[:, ht, b:b + 1],                 # lhsT [h, 1]
                                ets[ht][:, b, sc * SCW:(sc + 1) * SCW],  # rhs [h, s]
                                start=(ht == 0),
                                stop=(ht == HT - 1),
                                tile_position=(0, 32 * b),
                            )

                # softmax over s (free dim); both batches (partitions 0 and
                # 32) processed jointly in one instruction per stage
                prob_sb = spool.tile([33, S], f32)
                nmax = spool.tile([33, 1], f32)
                ssum = spool.tile([33, 1], f32)
                rinv = spool.tile([33, 1], f32)
                nc.vector.reduce_max(
                    nmax[:], ps[:], axis=mybir.AxisListType.X, negate=True
                )
                nc.scalar.activation(
                    out=prob_sb[:],
                    in_=ps[:],
                    func=mybir.ActivationFunctionType.Exp,
                    bias=nmax[:],
                    scale=1.0,
                    accum_out=ssum[:],
                )
                nc.vector.reciprocal(rinv[:], ssum[:])
                nc.vector.tensor_scalar_mul(
                    out=prob_sb[:], in0=prob_sb[:], scalar1=rinv[:]
                )
                # output on the scalar ring: a tail store must never block
                # the next rep's input stream on the sync ring
                for b in range(BL):
                    nc.scalar.dma_start(
                        out=probs[b:b + 1, :], in_=prob_sb[32 * b:32 * b + 1, :]
                    )

            if dynamic and reps > 1:
                assert reps % UNROLL == 0, (reps, UNROLL)
                with tc.For_i(0, reps // UNROLL, 1):
                    for _u in range(UNROLL):
                        emit_rep()
            else:
                for _rep in range(reps):
                    emit_rep()

    nc.compile()
    return nc


def get_nc(reps=1, dynamic=False):
    key = ("nc", reps, dynamic)
    if key not in _cached:
        _cached[key] = _build_nc(reps, dynamic)
    return _cached[key]


def prep_in_maps(encoder_output, last_decoder_state, W, b):
    enc16 = np.asarray(encoder_output, dtype=np.float32).astype(np.float16)  # [S,B,H]
    state = np.asarray(last_decoder_state, dtype=np.float32)[0, 0]           # [B,H]
    W32 = np.asarray(W, dtype=np.float32)
    # wtb[jt, p, it, j] = W[jt*128+j, it*128+p]
    wtb = np.ascontiguousarray(
        W32.astype(np.float16).reshape(HT, P, HT, P).transpose(0, 3, 2, 1)
    )
    bias = np.asarray(b, dtype=np.float32)
    bt = np.ascontiguousarray(bias.reshape(HT, P).T)                         # [p, jt]
    in_maps = []
    for c in range(NCORES):
        b0 = BL * c
        ec = enc16[:, b0:b0 + BL, :]                                         # [S,BL,H]
        enc_t = np.ascontiguousarray(ec.transpose(2, 1, 0)).reshape(HT, P, BL, S)
        stc = state[b0:b0 + BL, :].astype(np.float16)                        # [BL,H]
        st = np.ascontiguousarray(stc.reshape(BL, HT, P).transpose(2, 1, 0))
        in_maps.append({"enc_t": enc_t, "wtb": wtb, "st_in": st, "bt_in": bt})
    return in_maps


def assemble(results):
    out = np.empty((S, B), np.float32)
    for c in range(NCORES):
        out[:, BL * c:BL * (c + 1)] = results[c]["probs"].T
    return out[None, None]


def kernel(encoder_output, last_decoder_state, W, b):
    from concourse.bass_utils import run_bass_kernel_spmd

    nc = get_nc()
    in_maps = prep_in_maps(encoder_output, last_decoder_state, W, b)
    res = run_bass_kernel_spmd(nc, in_maps, core_ids=list(range(NCORES)))
    return assemble(res.results)
